# revision 3
# baseline (speedup 1.0000x reference)
"""Trainium2 Bass kernel for the dihedral-energy (gnn_message_passing) problem.

Strategy:
 - Shard the 2M dihedrals contiguously across 8 NeuronCores (250K each,
   padded to 128*1984).  pos/atom_types are fused into one [200000, 4] f32
   atom table (xyz + type bits); k1s/k2s are packed into a [390625, 16] f32
   coefficient table (9 used floats per row).  Both tables are replicated
   to all cores; mapping/mapping_batch are sharded.
 - On each core: gather the 4 atom rows per dihedral and the coefficient
   row (single-index-per-partition indirect DMAs - the only indirect-DMA
   form this hardware executes correctly), compute the dihedral angle via
   the normalization identity (no atan2/sin/cos needed - Chebyshev-style
   angle-addition recurrences on (cos t, sin t)), accumulate per-molecule
   energies with a sorted-window segmented reduction, and scatter into a
   per-core [128, 512] grid reduced across partitions with one matmul.
 - Host rebases molecule ids per shard and sums the per-core windows into
   the full [1024] energy vector (the all-reduce step).
NaN semantics match the reference: dihedrals with coincident middle atoms
(b2 = 0) produce NaN which poisons exactly their own molecule; b1 = 0 or
b3 = 0 degenerate dihedrals take the atan2(0,0) = 0 path.
"""
import sys

if '/opt/trn_rl_repo' not in sys.path:
    sys.path.insert(0, '/opt/trn_rl_repo')

import numpy as np

N_ATOMS = 200000
N_DIH = 2000000
N_TYPES = 25
N_MOL = 1024
N_CORES = 8
P = 128
F = 1984                 # free width per partition per core
MCORE = N_DIH // N_CORES             # 250000
MPAD = P * F                         # 253952
FT = 248                 # columns per compute tile
NT = F // FT             # 8 tiles
W = 512                  # per-core molecule window (scatter grid width)
PADMOL = 504             # local molecule slot for padding dihedrals

_compiled = None


def _build():
    import concourse.bass as bass
    import concourse.mybir as mybir
    import concourse.tile as tile
    from concourse import bacc

    AO = mybir.AluOpType
    ACT = mybir.ActivationFunctionType
    f32 = mybir.dt.float32
    i32 = mybir.dt.int32
    IOA = bass.IndirectOffsetOnAxis

    nc = bacc.Bacc('TRN2', target_bir_lowering=False, debug=False)
    atbl = nc.declare_dram_parameter('atbl', [N_ATOMS, 4], f32, isOutput=False)
    ctbl = nc.declare_dram_parameter('ctbl', [N_TYPES ** 4, 16], f32, isOutput=False)
    mapr = nc.declare_dram_parameter('mapr', [4, P, F], i32, isOutput=False)
    batch = nc.declare_dram_parameter('batch', [P, F], i32, isOutput=False)
    offb = nc.declare_dram_parameter('offb', [P, 8], i32, isOutput=False)
    energy = nc.declare_dram_parameter('energy', [1, W], f32, isOutput=True)

    with tile.TileContext(nc) as tc:
        with (
            tc.tile_pool(name='const', bufs=1) as cp,
            tc.tile_pool(name='head', bufs=2) as hp,
            tc.tile_pool(name='work', bufs=1) as wp,
            tc.tile_pool(name='dram', bufs=1, space='DRAM') as dp,
            tc.tile_pool(name='psum', bufs=1, space='PSUM') as pp,
        ):
            mt = [cp.tile([P, F], i32, tag=f'm{r}', name=f'mt{r}') for r in range(4)]
            for r in range(4):
                nc.sync.dma_start(mt[r][:], mapr[r, :, :])
            bt = cp.tile([P, F], i32)
            nc.sync.dma_start(bt[:], batch[:])
            ot = cp.tile([P, 8], i32)
            nc.sync.dma_start(ot[:], offb[:])
            zeros = cp.tile([P, FT], f32)
            nc.vector.memset(zeros[:], 0.0)
            nant = cp.tile([P, 8], f32)
            nc.vector.memset(nant[:], float('nan'))
            ones = cp.tile([P, 1], f32)
            nc.vector.memset(ones[:], 1.0)
            E8 = cp.tile([P, 8], f32)
            nc.vector.memset(E8[:], 0.0)
            E8n = cp.tile([P, 8], f32)
            nc.vector.memset(E8n[:], 0.0)
            b0 = bt[:, 0:1]

            for t in range(NT):
                sl = slice(t * FT, (t + 1) * FT)
                # --- gather 4 atom rows per dihedral ------------------
                gp = [hp.tile([P, FT, 4], f32, tag=f'g{r}', name=f'gp{r}') for r in range(4)]
                for r in range(4):
                    for k in range(FT):
                        nc.gpsimd.indirect_dma_start(
                            out=gp[r][:, k, :], out_offset=None, in_=atbl[:],
                            in_offset=IOA(ap=mt[r][:, t * FT + k:t * FT + k + 1],
                                          axis=0))
                # --- coefficient index ((t0*25+t1)*25+t2)*25+t3 -------
                it0 = wp.tile([P, FT], i32, tag='it0')
                it1 = hp.tile([P, FT], i32, tag='it1')
                nc.vector.scalar_tensor_tensor(
                    out=it0[:], in0=gp[0][:, :, 3].bitcast(i32), scalar=N_TYPES,
                    in1=gp[1][:, :, 3].bitcast(i32), op0=AO.mult, op1=AO.add)
                nc.vector.scalar_tensor_tensor(
                    out=it0[:], in0=it0[:], scalar=N_TYPES,
                    in1=gp[2][:, :, 3].bitcast(i32), op0=AO.mult, op1=AO.add)
                nc.vector.scalar_tensor_tensor(
                    out=it1[:], in0=it0[:], scalar=N_TYPES,
                    in1=gp[3][:, :, 3].bitcast(i32), op0=AO.mult, op1=AO.add)
                # --- gather coefficient rows (9 f32 of the 16-stride row)
                cf = hp.tile([P, FT, 9], f32, tag='cf')
                for k in range(FT):
                    nc.gpsimd.indirect_dma_start(
                        out=cf[:, k, :], out_offset=None, in_=ctbl[:],
                        in_offset=IOA(ap=it1[:, k:k + 1], axis=0))

                # --- bond vectors (SoA planes) ------------------------
                b1 = wp.tile([P, 3, FT], f32, tag='b1')
                b2 = wp.tile([P, 3, FT], f32, tag='b2')
                b3 = wp.tile([P, 3, FT], f32, tag='b3')
                for c in range(3):
                    nc.vector.tensor_tensor(out=b1[:, c, :], in0=gp[1][:, :, c],
                                            in1=gp[0][:, :, c], op=AO.subtract)
                    nc.vector.tensor_tensor(out=b2[:, c, :], in0=gp[2][:, :, c],
                                            in1=gp[1][:, :, c], op=AO.subtract)
                    nc.vector.tensor_tensor(out=b3[:, c, :], in0=gp[3][:, :, c],
                                            in1=gp[2][:, :, c], op=AO.subtract)

                # --- cross products n1 = b1 x b2, n2 = b2 x b3, mm = n1 x b2
                n1 = wp.tile([P, 3, FT], f32, tag='n1')
                n2 = wp.tile([P, 3, FT], f32, tag='n2')
                mm = wp.tile([P, 3, FT], f32, tag='mm')
                t1 = wp.tile([P, FT], f32, tag='t1')
                t2 = wp.tile([P, FT], f32, tag='t2')

                def cross(dst, a, b):
                    for (i, j, k) in ((0, 1, 2), (1, 2, 0), (2, 0, 1)):
                        nc.vector.tensor_tensor(out=t1[:], in0=a[:, j, :],
                                                in1=b[:, k, :], op=AO.mult)
                        nc.vector.tensor_tensor(out=t2[:], in0=a[:, k, :],
                                                in1=b[:, j, :], op=AO.mult)
                        nc.vector.tensor_tensor(out=dst[:, i, :], in0=t1[:],
                                                in1=t2[:], op=AO.subtract)

                cross(n1, b1, b2)
                cross(n2, b2, b3)
                cross(mm, n1, b2)

                # --- dots: x = n1.n2, yp = mm.n2, bb = b2.b2 ----------
                prod = wp.tile([P, 3, FT], f32, tag='prod')
                xy = wp.tile([P, 2, FT], f32, tag='xy')
                yp = wp.tile([P, FT], f32, tag='yp')
                bb = wp.tile([P, FT], f32, tag='bb')
                nc.vector.tensor_tensor(out=prod[:], in0=n1[:], in1=n2[:], op=AO.mult)
                nc.vector.tensor_reduce(out=xy[:, 0, :],
                                        in_=prod[:].rearrange('p n f -> p f n'),
                                        axis=mybir.AxisListType.X, op=AO.add)
                nc.vector.tensor_tensor(out=prod[:], in0=mm[:], in1=n2[:], op=AO.mult)
                nc.vector.tensor_reduce(out=yp[:],
                                        in_=prod[:].rearrange('p n f -> p f n'),
                                        axis=mybir.AxisListType.X, op=AO.add)
                nc.vector.tensor_tensor(out=prod[:], in0=b2[:], in1=b2[:], op=AO.mult)
                nc.vector.tensor_reduce(out=bb[:],
                                        in_=prod[:].rearrange('p n f -> p f n'),
                                        axis=mybir.AxisListType.X, op=AO.add)

                # --- y = yp * sqrt(1/bb)  (NaN when b2 == 0, like reference)
                rbb = wp.tile([P, FT], f32, tag='rbb')
                nc.vector.reciprocal(rbb[:], bb[:])
                srbb = wp.tile([P, FT], f32, tag='srbb')
                nc.scalar.activation(srbb[:], rbb[:], ACT.Sqrt)
                nc.vector.tensor_tensor(out=xy[:, 1, :], in0=yp[:], in1=srbb[:],
                                        op=AO.mult)

                # --- r2 = x^2 + y^2; guarded inverse sqrt -------------
                prodxy = wp.tile([P, 2, FT], f32, tag='prodxy')
                r2 = wp.tile([P, FT], f32, tag='r2')
                nc.vector.tensor_tensor(out=prodxy[:], in0=xy[:], in1=xy[:],
                                        op=AO.mult)
                nc.vector.tensor_reduce(out=r2[:],
                                        in_=prodxy[:].rearrange('p n f -> p f n'),
                                        axis=mybir.AxisListType.X, op=AO.add)
                ind = wp.tile([P, FT], f32, tag='ind')
                nc.vector.tensor_scalar(ind[:], r2[:], 1e-30, None, op0=AO.is_le)
                r2c = wp.tile([P, FT], f32, tag='r2c')
                nc.vector.tensor_scalar(r2c[:], r2[:], 1e-30, None, op0=AO.max)
                ir2 = wp.tile([P, FT], f32, tag='ir2')
                nc.vector.reciprocal(ir2[:], r2c[:])
                rxy = wp.tile([P, FT], f32, tag='rxy')
                nc.scalar.activation(rxy[:], ir2[:], ACT.Sqrt)

                # --- sin/cos ladder into S planes ---------------------
                # S planes: [1, sin1, cos1, sin2, cos2, sin3, cos3, sin4, cos4]
                S = wp.tile([P, 9, FT], f32, tag='S')
                nc.vector.memset(S[:, 0, :], 1.0)
                sin1 = S[:, 1, :]
                cos1 = S[:, 2, :]
                nc.vector.tensor_tensor(out=sin1, in0=xy[:, 1, :], in1=rxy[:],
                                        op=AO.mult)
                nc.vector.tensor_tensor(out=t1[:], in0=xy[:, 0, :], in1=rxy[:],
                                        op=AO.mult)
                nc.vector.tensor_tensor(out=cos1, in0=t1[:], in1=ind[:], op=AO.add)
                sin2 = S[:, 3, :]
                cos2 = S[:, 4, :]
                nc.vector.scalar_tensor_tensor(out=sin2, in0=sin1, scalar=2.0,
                                               in1=cos1, op0=AO.mult, op1=AO.mult)
                nc.vector.scalar_tensor_tensor(out=t1[:], in0=sin1, scalar=-2.0,
                                               in1=sin1, op0=AO.mult, op1=AO.mult)
                nc.vector.tensor_scalar(cos2, t1[:], 1.0, None, op0=AO.add)
                nc.vector.tensor_tensor(out=t1[:], in0=sin1, in1=cos2, op=AO.mult)
                nc.vector.tensor_tensor(out=t2[:], in0=cos1, in1=sin2, op=AO.mult)
                nc.vector.tensor_tensor(out=S[:, 5, :], in0=t1[:], in1=t2[:],
                                        op=AO.add)
                nc.vector.tensor_tensor(out=t1[:], in0=cos1, in1=cos2, op=AO.mult)
                nc.vector.tensor_tensor(out=t2[:], in0=sin1, in1=sin2, op=AO.mult)
                nc.vector.tensor_tensor(out=S[:, 6, :], in0=t1[:], in1=t2[:],
                                        op=AO.subtract)
                nc.vector.scalar_tensor_tensor(out=S[:, 7, :], in0=sin2, scalar=2.0,
                                               in1=cos2, op0=AO.mult, op1=AO.mult)
                nc.vector.scalar_tensor_tensor(out=t1[:], in0=cos2, scalar=2.0,
                                               in1=cos2, op0=AO.mult, op1=AO.mult)
                nc.vector.tensor_scalar(S[:, 8, :], t1[:], -1.0, None, op0=AO.add)

                # --- V = sum over 9 coef * S terms --------------------
                prodV = wp.tile([P, FT, 9], f32, tag='prodV')
                nc.vector.tensor_tensor(out=prodV[:], in0=cf[:],
                                        in1=S[:].rearrange('p n f -> p f n'),
                                        op=AO.mult)
                V = wp.tile([P, FT], f32, tag='V')
                nc.vector.tensor_reduce(out=V[:], in_=prodV[:],
                                        axis=mybir.AxisListType.X, op=AO.add)

                # --- segmented window accumulation --------------------
                u = wp.tile([P, FT], i32, tag='u')
                nc.vector.tensor_tensor(out=u[:], in0=bt[:, sl],
                                        in1=b0.to_broadcast([P, FT]),
                                        op=AO.subtract)
                nanm = wp.tile([P, FT], i32, tag='nanm')
                nc.vector.tensor_tensor(out=nanm[:], in0=V[:], in1=V[:],
                                        op=AO.not_equal)
                Vc = wp.tile([P, FT], f32, tag='Vc')
                nc.vector.tensor_copy(Vc[:], V[:])
                nc.vector.copy_predicated(Vc[:], nanm[:], zeros[:])
                e8t = wp.tile([P, 8], f32, tag='e8t')
                e8n = wp.tile([P, 8], f32, tag='e8n')
                msc = wp.tile([P, FT], f32, tag='msc')
                for w in range(8):
                    nc.vector.scalar_tensor_tensor(
                        out=msc[:], in0=u[:], scalar=w, in1=Vc[:],
                        op0=AO.is_equal, op1=AO.mult, accum_out=e8t[:, w:w + 1])
                for w in range(8):
                    nc.vector.scalar_tensor_tensor(
                        out=msc[:], in0=u[:], scalar=w, in1=nanm[:],
                        op0=AO.is_equal, op1=AO.mult, accum_out=e8n[:, w:w + 1])
                nc.vector.tensor_tensor(out=E8[:], in0=E8[:], in1=e8t[:], op=AO.add)
                nc.vector.tensor_tensor(out=E8n[:], in0=E8n[:], in1=e8n[:],
                                        op=AO.add)

            # --- tail: NaN injection, scatter, partition reduce -------
            nm = wp.tile([P, 8], i32, tag='nm')
            nc.vector.tensor_scalar(nm[:], E8n[:], 0.5, None, op0=AO.is_gt)
            E8f = wp.tile([P, 8], f32, tag='E8f')
            nc.vector.tensor_copy(E8f[:], E8[:])
            nc.vector.copy_predicated(E8f[:], nm[:], nant[:])
            offi = wp.tile([P, 8], i32, tag='offi')
            nc.vector.tensor_tensor(out=offi[:], in0=ot[:],
                                    in1=b0.to_broadcast([P, 8]), op=AO.add)
            e2d = dp.tile([P * W, 1], mybir.dt.float32)
            z512 = wp.tile([P, W], f32, tag='z512')
            nc.vector.memset(z512[:], 0.0)
            nc.sync.dma_start(e2d[:].rearrange('(p c) one -> p (c one)', p=P),
                              z512[:])
            for w in range(8):
                nc.gpsimd.indirect_dma_start(
                    out=e2d[:], out_offset=IOA(ap=offi[:, w:w + 1], axis=0),
                    in_=E8f[:, w:w + 1], in_offset=None)
            e2s = wp.tile([P, W], f32, tag='e2s')
            nc.sync.dma_start(e2s[:],
                              e2d[:].rearrange('(p c) one -> p (c one)', p=P))
            ps = pp.tile([1, W], f32)
            nc.tensor.matmul(ps[:], lhsT=ones[:], rhs=e2s[:], start=True, stop=True)
            eo = wp.tile([1, W], f32, tag='eo')
            nc.vector.tensor_copy(eo[:], ps[:])
            nc.sync.dma_start(energy[:], eo[:])

    nc.compile()
    return nc


def _get_compiled():
    global _compiled
    if _compiled is None:
        _compiled = _build()
    return _compiled


def _prep_inputs(pos, k1s, k2s, mapping, atom_types, mapping_batch):
    pos = np.ascontiguousarray(np.asarray(pos, dtype=np.float32))
    k1s = np.asarray(k1s, dtype=np.float32)
    k2s = np.asarray(k2s, dtype=np.float32)
    mapping = np.asarray(mapping).astype(np.int32)
    atom_types = np.asarray(atom_types).astype(np.int32)
    mapping_batch = np.asarray(mapping_batch).astype(np.int32)

    atbl = np.zeros((N_ATOMS, 4), np.float32)
    atbl[:, :3] = pos
    atbl[:, 3] = atom_types.view(np.float32)

    ctbl = np.zeros((N_TYPES ** 4, 16), np.float32)
    k1f = k1s.reshape(5, -1)
    k2f = k2s.reshape(5, -1)
    ctbl[:, 0] = k2f[0]
    for d in range(1, 5):
        ctbl[:, 2 * d - 1] = k1f[d]
        ctbl[:, 2 * d] = k2f[d]

    offb = (np.arange(P, dtype=np.int32)[:, None] * W
            + np.arange(8, dtype=np.int32)[None, :])

    in_maps = []
    bases = []
    spans = []
    padmap = np.tile(np.array([[0], [1], [2], [3]], np.int32), (1, MPAD - MCORE))
    for c in range(N_CORES):
        s = c * MCORE
        mchunk = mapping[:, s:s + MCORE]
        bchunk = mapping_batch[s:s + MCORE]
        base = int(bchunk[0])
        span = int(bchunk[-1]) - base + 1
        assert 0 < span <= 500, f'molecule span {span} exceeds window'
        bases.append(base)
        spans.append(span)
        mapr = np.concatenate([mchunk, padmap], axis=1).reshape(4, P, F)
        batch = np.concatenate(
            [bchunk - base, np.full(MPAD - MCORE, PADMOL, np.int32)]).reshape(P, F)
        in_maps.append({'atbl': atbl, 'ctbl': ctbl,
                        'mapr': np.ascontiguousarray(mapr),
                        'batch': np.ascontiguousarray(batch), 'offb': offb})
    return in_maps, bases, spans


LAST_EXEC_NS = None


def kernel(pos, k1s, k2s, mapping, atom_types, mapping_batch, trace=False):
    from concourse.bass_utils import run_bass_kernel_spmd
    global LAST_EXEC_NS

    nc = _get_compiled()
    in_maps, bases, spans = _prep_inputs(pos, k1s, k2s, mapping, atom_types,
                                         mapping_batch)
    kwargs = {}
    if trace:
        try:
            sys.path.insert(0, '/root/problem')
            import hwprof
            hwprof.install()
            kwargs['trace'] = True
        except Exception:
            pass
    res = None
    for attempt in range(3):
        try:
            res = run_bass_kernel_spmd(nc, in_maps, list(range(N_CORES)),
                                       **kwargs)
            break
        except Exception:
            if attempt == 2:
                raise
    LAST_EXEC_NS = getattr(res, 'exec_time_ns', None)
    full = np.zeros(N_MOL, np.float32)
    for c in range(N_CORES):
        e = res.results[c]['energy'][0]
        full[bases[c]:bases[c] + spans[c]] += e[:spans[c]]
    return full
